# revision 3
# baseline (speedup 1.0000x reference)
"""LSTM-Isoformer Trainium kernel: batch-sharded LSTM (8 rows/core) with
skewed 2-layer software pipeline + fused cell ops, AllGather of the fc1
output, iso-sharded fc2 with per-pair grouped softmax.
"""
import sys
for p in ("/opt/trn_rl_repo",):
    if p not in sys.path:
        sys.path.insert(0, p)
from contextlib import ExitStack
import numpy as np
import ml_dtypes

import concourse.bass as bass
import concourse.tile as tile
from concourse import bacc, mybir

B, S, H, ISO, NCORES = 64, 256, 256, 160000, 8
BLK = 512
BC = B // NCORES  # batch rows per core

BF = mybir.dt.bfloat16
F32 = mybir.dt.float32
AF = mybir.ActivationFunctionType
ALU = mybir.AluOpType


# ---------------- host-side layout / packing ----------------

def build_layout(gene_idx, n_genes):
    """Sort genes by run length, deal round-robin across cores, pack into
    uniform 512-slot blocks per length-bucket."""
    gene_idx = np.asarray(gene_idx).astype(np.int64)
    counts = np.bincount(gene_idx, minlength=n_genes)
    order = np.argsort(gene_idx, kind="stable")
    gene_starts = np.zeros(n_genes + 1, np.int64)
    np.cumsum(counts, out=gene_starts[1:])
    Ls = sorted(set(counts[counts > 0].tolist()))
    core_genes = [[[] for _ in range(NCORES)] for _ in Ls]
    for li, L in enumerate(Ls):
        genes_L = np.flatnonzero(counts == L)
        for j, g in enumerate(genes_L):
            core_genes[li][j % NCORES].append(g)
    buckets = []
    for li, L in enumerate(Ls):
        ng = max(len(core_genes[li][c]) for c in range(NCORES))
        gpb = BLK // L
        nblocks = (ng + gpb - 1) // gpb
        buckets.append(dict(L=L, ng=nblocks * gpb, gpb=gpb, nblocks=nblocks))
    NB = sum(b["nblocks"] for b in buckets)
    if NB % 2:
        buckets.append(dict(L=1, ng=BLK, gpb=BLK, nblocks=1))
        NB += 1
    ISO_C = NB * BLK
    slot_maps = np.full((NCORES, ISO_C), -1, np.int64)
    for c in range(NCORES):
        off = 0
        for li_b, b in enumerate(buckets):
            L, gpb, nblocks = b["L"], b["gpb"], b["nblocks"]
            glist = core_genes[li_b][c] if li_b < len(Ls) else []
            for bi in range(nblocks):
                base = off + bi * BLK
                for gi in range(gpb):
                    gidx = bi * gpb + gi
                    if gidx < len(glist):
                        g = glist[gidx]
                        iso = order[gene_starts[g]:gene_starts[g] + L]
                        slot_maps[c, base + gi * L: base + gi * L + L] = iso
            off += nblocks * BLK
    return buckets, slot_maps, NB, ISO_C


def reorder_scale_gates(W):
    """torch gate order i,f,g,o -> i,f,o,2g (factor 2 bakes the
    tanh(g) = 2*sigmoid(2g)-1 identity into the weights)."""
    i, f, g, o = np.split(np.asarray(W, np.float32), 4, axis=0)
    return np.concatenate([i, f, o, 2.0 * g], axis=0)


def lhsT_pack(WT, n_k, n_m):
    K, M = WT.shape
    a = WT.reshape(n_k, 128, n_m, 128).transpose(1, 0, 2, 3)
    return np.ascontiguousarray(a.reshape(128, n_k * n_m * 128))


def prep_all(inputs):
    ins = {k: np.asarray(v) for k, v in inputs.items()}
    n_genes = int(ins["n_genes"])
    buckets, slot_maps, NB, ISO_C = build_layout(ins["gene_idx"], n_genes)

    Whh0r = reorder_scale_gates(ins["Whh0"])
    Wih0r = reorder_scale_gates(ins["Wih0"])[:, 0]
    bias0r = reorder_scale_gates((ins["bih0"] + ins["bhh0"])[:, None])[:, 0]
    Whh1r = reorder_scale_gates(ins["Whh1"])
    Wih1r = reorder_scale_gates(ins["Wih1"])
    bias1r = reorder_scale_gates((ins["bih1"] + ins["bhh1"])[:, None])[:, 0]

    host = {}
    host["W0"] = lhsT_pack(Whh0r.T, 2, 8).astype(ml_dtypes.bfloat16)
    comb1 = np.concatenate([Whh1r, Wih1r], axis=1)     # [1024, 512]
    host["W1"] = lhsT_pack(comb1.T, 4, 8).astype(ml_dtypes.bfloat16)
    host["WFC"] = lhsT_pack(np.asarray(ins["W1"], np.float32).T, 2, 2).astype(ml_dtypes.bfloat16)
    host["b1T"] = np.ascontiguousarray(
        np.asarray(ins["b1"], np.float32).reshape(2, 128).T).astype(np.float32)
    # layer0 const lhsT: row0 = Wih0 column, row1 = bias0 (both gate-reordered/scaled)
    host["WC0"] = np.stack([Wih0r, bias0r], axis=0).astype(ml_dtypes.bfloat16)   # [2, 1024]
    host["WC1"] = np.stack([np.zeros_like(bias1r), bias1r], axis=0).astype(ml_dtypes.bfloat16)
    # per-core x const rhs: [2, S*BC]: row0 = x rows of this core (t-major), row1 = ones
    x = np.asarray(ins["x"], np.float32)               # [B, S]
    XCs = []
    for c in range(NCORES):
        xc = np.ones((2, S * BC), np.float32)
        xc[0] = x[c * BC:(c + 1) * BC, :].T.reshape(-1)   # [t, b] flattened
        XCs.append(xc.astype(ml_dtypes.bfloat16))
    host["XCs"] = XCs

    W2 = np.asarray(ins["W2"], np.float32)
    b2 = np.asarray(ins["b2"], np.float32)
    W2TD, B2P = [], []
    for c in range(NCORES):
        sm = slot_maps[c]
        W2P = np.where(sm[:, None] >= 0, W2[np.maximum(sm, 0)], 0.0)
        b2P = np.where(sm >= 0, b2[np.maximum(sm, 0)], 0.0)
        t = W2P.T.reshape(2, 128, ISO_C).transpose(1, 0, 2)
        W2TD.append(np.ascontiguousarray(t).astype(ml_dtypes.bfloat16))
        B2P.append(b2P.astype(np.float32))
    host["W2TD"] = W2TD
    host["B2P"] = B2P
    host["buckets"] = buckets
    host["slot_maps"] = slot_maps
    host["NB"] = NB
    host["ISO_C"] = ISO_C
    return host


# ---------------- device kernel ----------------

def build(buckets, NB, ISO_C, S_steps=S, debug_hid=False, debug_state=False):
    NPAIR = NB // 2
    nc = bacc.Bacc("TRN2", target_bir_lowering=False, debug=False, enable_asserts=False)
    d_hdbg = (nc.dram_tensor("hdbg", [128, 2 * 64], F32, kind="ExternalOutput").ap()
              if debug_hid else None)
    d_sdbg = (nc.dram_tensor("sdbg", [128, 8 * BC], F32, kind="ExternalOutput").ap()
              if debug_state else None)

    d_w0 = nc.dram_tensor("w0", [128, 2 * 1024], BF, kind="ExternalInput").ap()
    d_w1 = nc.dram_tensor("w1", [128, 4 * 1024], BF, kind="ExternalInput").ap()
    d_wc0 = nc.dram_tensor("wc0", [2, 1024], BF, kind="ExternalInput").ap()
    d_wc1 = nc.dram_tensor("wc1", [2, 1024], BF, kind="ExternalInput").ap()
    d_xc = nc.dram_tensor("xc", [2, S * BC], BF, kind="ExternalInput").ap()
    d_wfc = nc.dram_tensor("wfc", [128, 2 * 256], BF, kind="ExternalInput").ap()
    d_b1t = nc.dram_tensor("b1t", [128, 2], F32, kind="ExternalInput").ap()
    d_w2 = nc.dram_tensor("w2t", [128, 2, ISO_C], BF, kind="ExternalInput").ap()
    d_b2 = nc.dram_tensor("b2p", [1, ISO_C], BF, kind="ExternalInput").ap()
    d_hidloc = nc.dram_tensor("hidloc", [128, 2 * BC], BF, kind="Internal").ap()
    d_hidg = nc.dram_tensor("hidg", [NCORES, 128, 2 * BC], BF, kind="Internal",
                            addr_space="Shared").ap()
    d_out = nc.dram_tensor("out", [B, ISO_C], F32, kind="ExternalOutput").ap()

    ctx = ExitStack()
    with ctx:
        tc = ctx.enter_context(tile.TileContext(nc, trace_sim=False))
        const = ctx.enter_context(tc.tile_pool(name="const", bufs=1))
        w2pre_pool = ctx.enter_context(tc.tile_pool(name="w2pre", bufs=1))
        st = ctx.enter_context(tc.tile_pool(name="state", bufs=2))
        tmp = ctx.enter_context(tc.tile_pool(name="tmp", bufs=3))
        ex_pool = ctx.enter_context(tc.tile_pool(name="ex", bufs=4))
        den_pool = ctx.enter_context(tc.tile_pool(name="den", bufs=4))
        psl = ctx.enter_context(tc.tile_pool(name="psl", bufs=2, space="PSUM"))
        ps_f = ctx.enter_context(tc.tile_pool(name="psf", bufs=4, space="PSUM"))

        # ---- weight preloads ----
        w0 = const.tile([128, 2048], BF)
        nc.sync.dma_start(w0[:], d_w0)
        w1 = const.tile([128, 4096], BF)
        nc.sync.dma_start(w1[:], d_w1)
        wc0 = const.tile([2, 1024], BF)
        nc.sync.dma_start(wc0[:], d_wc0)
        wc1 = const.tile([2, 1024], BF)
        nc.sync.dma_start(wc1[:], d_wc1)
        xc = const.tile([2, S * BC], BF)
        nc.sync.dma_start(xc[:], d_xc)
        wfc = const.tile([128, 512], BF)
        nc.sync.dma_start(wfc[:], d_wfc)
        b1t = const.tile([128, 2], F32)
        nc.sync.dma_start(b1t[:], d_b1t)
        ones64 = const.tile([1, 64], BF)
        nc.vector.memset(ones64[:], 1.0)

        # W2/b2 full prestream into SBUF (overlaps the LSTM)
        w2pre = w2pre_pool.tile([128, 2, NPAIR * 1024], BF)
        for q in range(NPAIR):
            nc.sync.dma_start(w2pre[:, :, q * 1024:(q + 1) * 1024],
                              d_w2[:, :, q * 1024:(q + 1) * 1024])
        b2pre = w2pre_pool.tile([1, ISO_C], BF)
        nc.sync.dma_start(b2pre[:], d_b2)

        # ---- LSTM state ----
        h0 = st.tile([128, 2, BC], BF, tag="h0")
        c0 = st.tile([128, 2 * BC], F32, tag="c0")
        h1 = st.tile([128, 2, BC], BF, tag="h1")
        c1 = st.tile([128, 2 * BC], F32, tag="c1")
        for t_ in (h0, c0, h1, c1):
            nc.vector.memset(t_[:], 0.0)

        MM_ORDER = [6, 7, 0, 1, 2, 3, 4, 5]   # g-tiles finish first

        def mm_gates(psum, wc, w, nkt, rhs_fn, t):
            # one PSUM accumulation group at a time (m-outer): interleaved
            # open groups corrupt PSUM on this hw
            for m in MM_ORDER:
                nc.tensor.matmul(psum[:, m * BC:(m + 1) * BC],
                                 lhsT=wc[:, m * 128:(m + 1) * 128],
                                 rhs=xc[:, t * BC:(t + 1) * BC],
                                 start=True, stop=False)
                for kt in range(nkt):
                    nc.tensor.matmul(psum[:, m * BC:(m + 1) * BC],
                                     lhsT=w[:, kt * 1024 + m * 128:(m + 1) * 128 + kt * 1024],
                                     rhs=rhs_fn(kt), start=False, stop=(kt == nkt - 1))

        def sig(pg, tag):
            # sg cols: [sig(i) sig(f) sig(o) sig(2g)]
            sg = tmp.tile([128, 8 * BC], F32, tag=tag)
            nc.scalar.activation(sg[:], pg[:], AF.Sigmoid)
            return sg

        def cell(sg, c_prev, tagc):
            # all on DVE back-to-back: t2 = sig_f * c_prev;
            # t1 = (sig_2g - 0.5) * sig_i; c = 2*t1 + t2 = sig_f*c + sig_i*tanh(g)
            t2 = tmp.tile([128, 2 * BC], F32, tag=tagc + "t2")
            nc.vector.tensor_tensor(out=t2[:], in0=sg[:, 2 * BC:4 * BC],
                                    in1=c_prev[:], op=ALU.mult)
            t1 = tmp.tile([128, 2 * BC], F32, tag=tagc + "t1")
            nc.vector.scalar_tensor_tensor(out=t1[:], in0=sg[:, 6 * BC:8 * BC],
                                           scalar=0.5, in1=sg[:, 0:2 * BC],
                                           op0=ALU.subtract, op1=ALU.mult)
            c = st.tile([128, 2 * BC], F32, tag=tagc)
            nc.vector.scalar_tensor_tensor(out=c[:], in0=t1[:], scalar=2.0,
                                           in1=t2[:], op0=ALU.mult, op1=ALU.add)
            return c

        def cell_tail(sg, c, tagh, tagth):
            th = tmp.tile([128, 2 * BC], F32, tag=tagth)
            nc.scalar.activation(th[:], c[:], AF.Tanh)
            h = st.tile([128, 2, BC], BF, tag=tagh)
            nc.vector.tensor_tensor(out=h[:].rearrange("p k b -> p (k b)"),
                                    in0=sg[:, 4 * BC:6 * BC], in1=th[:], op=ALU.mult)
            return h

        # ---- skewed pipeline: tick t = layer0 step t + layer1 step t-1 ----
        for t in range(S_steps + 1):
            h0prev = h0
            if t < S_steps:
                pg0 = psl.tile([128, 8 * BC], F32, tag="pg0")
                mm_gates(pg0, wc0, w0, 2, lambda kt: h0prev[:, kt, :], t)
            if t >= 1:
                pg1 = psl.tile([128, 8 * BC], F32, tag="pg1")
                mm_gates(pg1, wc1, w1, 4,
                         lambda kt: (h1[:, kt, :] if kt < 2 else h0prev[:, kt - 2, :]),
                         t - 1)
            if t < S_steps:
                sg0 = sig(pg0, "sg0")
            if t >= 1:
                sg1 = sig(pg1, "sg1")
            if t < S_steps:
                c0 = cell(sg0, c0, "c0")
                h0 = cell_tail(sg0, c0, "h0", "th0")
            if t >= 1:
                c1 = cell(sg1, c1, "c1")
                h1 = cell_tail(sg1, c1, "h1", "th1")

        if debug_state:
            sdbg = const.tile([128, 8 * BC], F32)
            nc.scalar.activation(sdbg[:, 0:2 * BC],
                                 h0[:].rearrange("p k b -> p (k b)"), AF.Copy)
            nc.vector.tensor_copy(out=sdbg[:, 2 * BC:4 * BC], in_=c0[:])
            nc.scalar.activation(sdbg[:, 4 * BC:6 * BC],
                                 h1[:].rearrange("p k b -> p (k b)"), AF.Copy)
            nc.vector.tensor_copy(out=sdbg[:, 6 * BC:8 * BC], in_=c1[:])
            nc.sync.dma_start(d_sdbg, sdbg[:])

        # ---- fc1 on local batch rows ----
        pf = psl.tile([128, 2 * BC], F32, tag="pg0")
        for m in range(2):
            for kt in range(2):
                nc.tensor.matmul(pf[:, m * BC:(m + 1) * BC],
                                 lhsT=wfc[:, kt * 256 + m * 128:(m + 1) * 128 + kt * 256],
                                 rhs=h1[:, kt, :], start=(kt == 0), stop=(kt == 1))
        hidloc = const.tile([128, 2, BC], BF)
        for m in range(2):
            nc.scalar.activation(hidloc[:, m, :], pf[:, m * BC:(m + 1) * BC],
                                 AF.Relu, bias=b1t[:, m:m + 1])

        # ---- allgather hid across the 8 cores ----
        nc.sync.dma_start(d_hidloc, hidloc[:].rearrange("p k b -> p (k b)"))
        nc.gpsimd.collective_compute(
            "AllGather", ALU.bypass,
            replica_groups=[list(range(NCORES))],
            ins=[d_hidloc], outs=[d_hidg],
        )
        hid = const.tile([128, 2, NCORES, BC], BF)   # [p, kt, core, b] = [128, 2, 64]
        nc.sync.dma_start(hid[:], d_hidg.rearrange("c p (k b) -> p k c b", k=2))
        if debug_hid:
            hdbg = const.tile([128, 2 * 64], F32)
            nc.scalar.activation(hdbg[:], hid[:].rearrange("p k c b -> p (k c b)"),
                                 AF.Copy)
            nc.sync.dma_start(d_hdbg, hdbg[:])

        # ---- fc2 + exp + per-pair grouped softmax + store ----
        blocks = []
        for bk in buckets:
            blocks.extend([(bk["L"], bk["gpb"])] * bk["nblocks"])
        assert len(blocks) == NB

        for q in range(NPAIR):
            w2q = w2pre[:, :, q * 1024:(q + 1) * 1024]
            pl = ps_f.tile([128, 512], F32, tag="pl")
            for hh in range(2):
                tp = (0, 64) if hh == 1 else None
                out_ap = pl[hh * 64:(hh + 1) * 64, :]
                for kt in range(2):
                    nc.tensor.matmul(
                        out_ap, lhsT=hid[:, kt, :].rearrange("p c b -> p (c b)"),
                        rhs=w2q[:, kt, hh * 512:(hh + 1) * 512],
                        start=(kt == 0), stop=False, tile_position=tp)
                nc.tensor.matmul(
                    out_ap, lhsT=ones64[:],
                    rhs=b2pre[:, q * 1024 + hh * 512:q * 1024 + (hh + 1) * 512],
                    start=False, stop=True, tile_position=tp)
            ex = ex_pool.tile([128, 512], F32, tag="ex")
            nc.scalar.activation(ex[:], pl[:], AF.Exp)
            # grouped softmax on both halves of this pair
            for hh in range(2):
                L, gpb = blocks[2 * q + hh]
                prow = slice(hh * 64, hh * 64 + 64)
                if L == 1:
                    nc.vector.memset(ex[prow, :], 1.0)
                    continue
                exg = ex[prow, 0:gpb * L].rearrange("p (g l) -> p g l", g=gpb)
                den = den_pool.tile([128, 256], F32, tag="den")
                dn = den[prow, 0:gpb]
                nc.vector.tensor_reduce(out=dn, in_=exg, axis=mybir.AxisListType.X,
                                        op=ALU.add)
                nc.vector.reciprocal(out=dn, in_=dn)
                bcast = den[prow, 0:gpb].rearrange("p (g o) -> p g o", o=1).to_broadcast(
                    [64, gpb, L])
                nc.vector.tensor_tensor(out=exg, in0=exg, in1=bcast, op=ALU.mult)
            for hh in range(2):
                nc.sync.dma_start(
                    d_out[:, q * 1024 + hh * 512:q * 1024 + (hh + 1) * 512],
                    ex[hh * 64:(hh + 1) * 64, :])

    nc.compile()
    return nc


def make_in_map(host, core):
    return {
        "w0": host["W0"], "w1": host["W1"],
        "wc0": host["WC0"], "wc1": host["WC1"],
        "xc": host["XCs"][core],
        "wfc": host["WFC"], "b1t": host["b1T"],
        "w2t": host["W2TD"][core],
        "b2p": host["B2P"][core].astype(ml_dtypes.bfloat16).reshape(1, -1),
    }


def build_for_timing(host):
    return build(host["buckets"], host["NB"], host["ISO_C"], S_steps=S)


def kernel(**inputs):
    ins = {}
    for k, v in inputs.items():
        ins[k] = np.asarray(v) if not np.isscalar(v) else v
    host = prep_all(ins)
    nc = build(host["buckets"], host["NB"], host["ISO_C"], S_steps=S)
    from concourse import bass_utils
    in_maps = [make_in_map(host, c) for c in range(NCORES)]
    res = bass_utils.run_bass_kernel_spmd(nc, in_maps, core_ids=list(range(NCORES)))
    full = np.zeros((B, ISO), np.float32)
    for c in range(NCORES):
        sm = host["slot_maps"][c]
        valid = sm >= 0
        full[:, sm[valid]] = res.results[c]["out"][:, valid]
    return full
